# revision 42
# baseline (speedup 1.0000x reference)
"""MoE (8 experts, top-2) Trainium2 kernel, expert-parallel across 8 NeuronCores.

Strategy:
  - Each core owns one expert (weights sharded along the expert axis; gate
    replicated). Everything data-dependent runs on device:
      * router logits (bf16 hi/lo split, fp32 accuracy), token-major streamed
        in 2 halves so top-2 for half 0 overlaps the stream of half 1
      * top-2 + renormalized gate weights (DVE), per-expert token compaction
        (gpsimd sparse_gather)
      * compacted (tokid, coef) relayout via one PE transpose + SBUF->SBUF
        DMAs with contiguous runs (no element-granular DRAM round trip)
      * token dispatch (indirect DMA gather of selected token rows) + PE
        transposes to [H, cap]
      * expert MLP GEMMs in bf16: (silu(x@w1) * (x@w3)) @ w2, scaled by the
        gate coefficient
  - Weight streams are gated behind the x-transpose stream so the router
    phase runs at full HBM bandwidth.
  - Each core returns its expert's (transposed, bf16) token outputs + the
    compacted token index list; the host scatter-adds the 8 partial outputs.
"""
import sys

sys.path.insert(0, "/opt/trn_rl_repo")

import numpy as np

T, H, II, E = 2048, 1024, 4096, 8
P = 128
NT = T // P          # 16 token tiles
HC = H // P          # 8 hidden chunks
IC = II // P         # 32 intermediate chunks
NCORES = 8
NHALF = 2            # router stream halves
TH = T // NHALF      # tokens per half
NTH = TH // P        # token tiles per half


def _groups(cap):
    """Split cap token columns into PSUM-sized GEMM groups (<=512 each).
    First group is 256 (= 2 gather tiles) so it can start earliest."""
    gs, off = [], 0
    first = min(256, cap)
    gs.append((0, first))
    off = first
    while off < cap:
        n = min(512, cap - off)
        gs.append((off, n))
        off += n
    return gs


_build_cache = {}


def _build(cap):
    """Build + schedule the per-core Tile kernel for token capacity `cap`."""
    import concourse.bass as bass
    import concourse.bacc as bacc
    import concourse.mybir as mybir
    from concourse.tile import TileContext
    from concourse.tile_rust import add_dep_helper

    f32 = mybir.dt.float32
    i32 = mybir.dt.int32
    u32 = mybir.dt.uint32
    u8 = mybir.dt.uint8
    bf16 = mybir.dt.bfloat16
    AF = mybir.ActivationFunctionType
    OP = mybir.AluOpType

    assert cap % 16 == 0
    cf = cap // 16       # columns of [16, cf] compacted layout
    nft = cap // P       # full 128-token gather tiles
    rem = cap - nft * P  # remainder gather tile rows
    ntile = nft + (1 if rem else 0)
    groups = _groups(cap)
    cfm = (cf // 8) * 8  # vt rows covered by full gather tiles * 8

    nc = bacc.Bacc("TRN2", target_bir_lowering=False)

    # ---- I/O ----
    f8 = mybir.dt.float8e4
    xth = nc.declare_dram_parameter("xth", [H, T], bf16, isOutput=False)
    # low-order residual of x, scaled by 2^7, in fp8 (verified: 0 top-2 flips)
    xtl8 = nc.declare_dram_parameter("xtl8", [H, T], f8, isOutput=False)
    x = nc.declare_dram_parameter("x", [T, H], bf16, isOutput=False)
    gwh = nc.declare_dram_parameter("gwh", [H, E], bf16, isOutput=False)
    gwl = nc.declare_dram_parameter("gwl", [H, E], bf16, isOutput=False)
    gw8d = nc.declare_dram_parameter("gw8", [H, E], f8, isOutput=False)  # 16*g in fp8
    # w1/w3 host-blocked: [icg*128+p, hc*512+q] = w1[hc*128+p, (icg*4+q//128)*128+q%128]
    # -> one [128, 4096] DMA per icg with 8KB contiguous per-partition lines
    w1 = nc.declare_dram_parameter("w1", [1024, 4096], bf16, isOutput=False)
    w3 = nc.declare_dram_parameter("w3", [1024, 4096], bf16, isOutput=False)
    # w2 host-blocked: [hc*128+p, ic*128+h] = w2[ic*128+p, hc*128+h]
    w2 = nc.declare_dram_parameter("w2", [1024, 4096], bf16, isOutput=False)
    oh = nc.declare_dram_parameter("oh", [P, NT * E], f32, isOutput=False)
    tokid1 = nc.declare_dram_parameter("tokid1", [P, NT], f32, isOutput=False)
    slotg_d = nc.declare_dram_parameter("slotg", [16, cf], f32, isOutput=False)
    ident = nc.declare_dram_parameter("ident", [P, P], f32, isOutput=False)

    o_yt = nc.declare_dram_parameter("o_yt", [H, cap], bf16, isOutput=True)
    o_idx = nc.declare_dram_parameter("o_idx", [cap], i32, isOutput=True)
    o_cnt = nc.declare_dram_parameter("o_cnt", [1, 1], u32, isOutput=True)

    with TileContext(nc) as tc:
        with (
            tc.tile_pool(name="sb", bufs=1) as sb,
            tc.tile_pool(name="sbw", bufs=2) as sbw,
            tc.tile_pool(name="psum", bufs=2, space="PSUM") as psg,
        ):
            # ---- constants (scalar HWDGE ring; small) ----
            idt = sb.tile([P, P], f32, tag="idt")
            nc.scalar.dma_start(out=idt[:], in_=ident[:])
            idtb = sb.tile([P, P], bf16, tag="idtb")
            nc.vector.tensor_copy(out=idtb[:], in_=idt[:])
            gw_h = sb.tile([P, HC * E], bf16, tag="gwh")
            nc.scalar.dma_start(
                out=gw_h[:].rearrange("p (hc e) -> p hc e", e=E),
                in_=gwh[:].rearrange("(hc p) e -> p hc e", p=P),
            )
            gw_l = sb.tile([P, HC * E], bf16, tag="gwl")
            nc.scalar.dma_start(
                out=gw_l[:].rearrange("p (hc e) -> p hc e", e=E),
                in_=gwl[:].rearrange("(hc p) e -> p hc e", p=P),
            )
            gw_8 = sb.tile([P, HC * E], f8, tag="gw8")
            nc.scalar.dma_start(
                out=gw_8[:].rearrange("p (hc e) -> p hc e", e=E),
                in_=gw8d[:].rearrange("(hc p) e -> p hc e", p=P),
            )

            # PE warm-up: flip HAM to full clock before the first router MMs
            scr = sb.tile([P, 512], bf16, tag="scr")
            nc.vector.memset(scr[:], 0.0)
            for wi in range(8):
                wps = psg.tile([P, 512], f32, tag="mm2", name=f"warm0_{wi}")
                nc.tensor.matmul(out=wps[:], lhsT=idtb[:], rhs=scr[:],
                                 start=True, stop=True)
            # ---- A. router: logitsT [8, 2048], token-major in 2 halves ----
            logitsT = sb.tile([E, T], f32, tag="logitsT")
            l_all = sb.tile([P, NT * E], f32, tag="l_all")
            l3 = l_all[:].rearrange("p (t e) -> p t e", e=E)

            # per-half top-2 intermediates
            m1 = sb.tile([P, NT, 1], f32, tag="m1")
            m2 = sb.tile([P, NT, 1], f32, tag="m2")
            wt1 = sb.tile([P, NT], f32, tag="wt1")
            wt2 = sb.tile([P, NT], f32, tag="wt2")
            coef = sb.tile([P, NT], f32, tag="coef")
            selm = sb.tile([P, NT], f32, tag="selm")
            enc = sb.tile([P, NT], f32, tag="enc")

            # big consts via the (idle) gpsimd SWDGE ring — keeps the two
            # HWDGE rings free for the x stream
            oh_sb = sb.tile([P, NT * E], f32, tag="oh")
            nc.gpsimd.dma_start(out=oh_sb[:], in_=oh[:])
            tk1 = sb.tile([P, NT], f32, tag="tk1")
            nc.gpsimd.dma_start(out=tk1[:], in_=tokid1[:])
            slotg = sb.tile([16, cf], f32, tag="slotg")
            nc.gpsimd.dma_start(out=slotg[:], in_=slotg_d[:])

            x_dma_insts = []
            for half in range(NHALF):
                t0 = half * TH
                ts = slice(t0, t0 + TH)
                nq = TH // 512  # 512-col psum groups in this half
                ps_l = [
                    psg.tile([E, 512], f32, tag=f"mm{q}", name=f"psl{half}{q}")
                    for q in range(nq)
                ]
                ps_8 = [
                    psg.tile([E, 512], f32, tag=f"mm{q + 2}", name=f"ps8{half}{q}")
                    for q in range(nq)
                ]
                for c4 in range(HC // 4):
                    # 4 hidden chunks per DMA: 1MB hi (bf16) + 0.5MB lo (fp8);
                    # cross-assign rings so each carries ~1.5MB per half
                    e_h = nc.sync if c4 == 0 else nc.scalar
                    e_l = nc.scalar if c4 == 0 else nc.sync
                    xt_h = sbw.tile([P, 4 * TH], bf16, tag="xth", bufs=2)
                    ih = e_h.dma_start(
                        out=xt_h[:].rearrange("p (c t) -> p c t", c=4),
                        in_=xth[c4 * 4 * P:(c4 + 1) * 4 * P, ts].rearrange(
                            "(c p) t -> p c t", p=P),
                    )
                    xt_l = sbw.tile([P, 4 * TH], f8, tag="xtl", bufs=2)
                    il = e_l.dma_start(
                        out=xt_l[:].rearrange("p (c t) -> p c t", c=4),
                        in_=xtl8[c4 * 4 * P:(c4 + 1) * 4 * P, ts].rearrange(
                            "(c p) t -> p c t", p=P),
                    )
                    x_dma_insts.append(ih)
                    x_dma_insts.append(il)
                    for ci in range(4):
                        hc = 4 * c4 + ci
                        for q in range(nq):
                            qs = slice(ci * TH + q * 512, ci * TH + (q + 1) * 512)
                            st = (hc == 0)
                            sp = (hc == HC - 1)
                            nc.tensor.matmul(
                                out=ps_l[q][:],
                                lhsT=gw_h[:, hc * E:(hc + 1) * E],
                                rhs=xt_h[:, qs],
                                start=st, stop=False,
                            )
                            nc.tensor.matmul(
                                out=ps_l[q][:],
                                lhsT=gw_l[:, hc * E:(hc + 1) * E],
                                rhs=xt_h[:, qs],
                                start=False, stop=sp,
                            )
                            nc.tensor.matmul(
                                out=ps_8[q][:],
                                lhsT=gw_8[:, hc * E:(hc + 1) * E],
                                rhs=xt_l[:, qs],
                                start=st, stop=sp,
                            )
                for q in range(nq):
                    # logits = ps_l + 2^-11 * ps_8  (fp8 term carries 2^11 scale)
                    t8 = sbw.tile([E, 512], f32, tag="t8")
                    nc.scalar.activation(out=t8[:], in_=ps_8[q][:],
                                         func=AF.Copy, scale=2.0 ** -11)
                    nc.vector.tensor_add(
                        out=logitsT[:, t0 + q * 512:t0 + (q + 1) * 512],
                        in0=ps_l[q][:], in1=t8[:],
                    )

                # ---- B. transpose this half's logits into l_all ----
                for ci in range(half * NTH, (half + 1) * NTH):
                    tp = psg.tile([P, E], f32, tag="mm3", name=f"ltp{ci}")
                    nc.tensor.transpose(
                        out=tp[:],
                        in_=logitsT[:, ci * P:(ci + 1) * P],
                        identity=idt[0:E, 0:E],
                    )
                    nc.vector.tensor_copy(out=l_all[:, ci * E:(ci + 1) * E], in_=tp[:])

                # ---- C. top-2 + coef for this half ----
                hs = slice(half * NTH, (half + 1) * NTH)
                l3h = l3[:, hs, :]
                nh = NTH
                m1a = sb.tile([P, nh, 4], f32, tag=f"m1a{half}")
                m2a = sb.tile([P, nh, 4], f32, tag=f"m2a{half}")
                nc.vector.tensor_tensor(out=m1a[:], in0=l3h[:, :, 0::2], in1=l3h[:, :, 1::2], op=OP.max)
                nc.vector.tensor_tensor(out=m2a[:], in0=l3h[:, :, 0::2], in1=l3h[:, :, 1::2], op=OP.min)
                m1b = sb.tile([P, nh, 2], f32, tag=f"m1b{half}")
                m2b = sb.tile([P, nh, 2], f32, tag=f"m2b{half}")
                tmin = sb.tile([P, nh, 2], f32, tag=f"tmin{half}")
                nc.vector.tensor_tensor(out=m1b[:], in0=m1a[:, :, 0::2], in1=m1a[:, :, 1::2], op=OP.max)
                nc.vector.tensor_tensor(out=tmin[:], in0=m1a[:, :, 0::2], in1=m1a[:, :, 1::2], op=OP.min)
                nc.vector.tensor_tensor(out=m2b[:], in0=m2a[:, :, 0::2], in1=m2a[:, :, 1::2], op=OP.max)
                nc.vector.tensor_tensor(out=m2b[:], in0=m2b[:], in1=tmin[:], op=OP.max)
                m1h = m1[:, hs, :]
                m2h = m2[:, hs, :]
                tmin2 = sb.tile([P, nh, 1], f32, tag=f"tmin2{half}")
                nc.vector.tensor_tensor(out=m1h, in0=m1b[:, :, 0:1], in1=m1b[:, :, 1:2], op=OP.max)
                nc.vector.tensor_tensor(out=tmin2[:], in0=m1b[:, :, 0:1], in1=m1b[:, :, 1:2], op=OP.min)
                nc.vector.tensor_tensor(out=m2h, in0=m2b[:, :, 0:1], in1=m2b[:, :, 1:2], op=OP.max)
                nc.vector.tensor_tensor(out=m2h, in0=m2h, in1=tmin2[:], op=OP.max)

                dq = sb.tile([P, nh], f32, tag=f"dq{half}")
                nc.vector.tensor_sub(out=dq[:], in0=m2[:, hs, 0], in1=m1[:, hs, 0])
                q_t = sb.tile([P, nh], f32, tag=f"q{half}")
                nc.scalar.activation(out=q_t[:], in_=dq[:], func=AF.Exp)
                s_t = sb.tile([P, nh], f32, tag=f"s{half}")
                nc.vector.tensor_scalar_add(s_t[:], q_t[:], 1.0)
                nc.vector.reciprocal(wt1[:, hs], s_t[:])
                nc.vector.tensor_mul(out=wt2[:, hs], in0=q_t[:], in1=wt1[:, hs])

                le_m = sb.tile([P, nh, E], f32, tag=f"lem{half}")
                nc.vector.tensor_mul(
                    out=le_m[:], in0=l3h,
                    in1=oh_sb[:].rearrange("p (t e) -> p t e", e=E)[:, hs, :],
                )
                le = sb.tile([P, nh], f32, tag=f"le{half}")
                nc.vector.reduce_sum(
                    out=le[:].rearrange("p (t o) -> p t o", o=1),
                    in_=le_m[:],
                    axis=mybir.AxisListType.X,
                )
                eq1 = sb.tile([P, nh], f32, tag=f"eq1{half}")
                eq2 = sb.tile([P, nh], f32, tag=f"eq2{half}")
                nc.vector.tensor_tensor(out=eq1[:], in0=le[:], in1=m1[:, hs, 0], op=OP.is_equal)
                nc.vector.tensor_tensor(out=eq2[:], in0=le[:], in1=m2[:, hs, 0], op=OP.is_equal)
                t1 = sb.tile([P, nh], f32, tag=f"t1{half}")
                nc.vector.tensor_mul(out=coef[:, hs], in0=eq1[:], in1=wt1[:, hs])
                nc.vector.tensor_mul(out=t1[:], in0=eq2[:], in1=wt2[:, hs])
                nc.vector.tensor_add(out=coef[:, hs], in0=coef[:, hs], in1=t1[:])
                nc.vector.tensor_add(out=selm[:, hs], in0=eq1[:], in1=eq2[:])

                # enc = (tokid + 1 + min(coef,.999)/2) * selm - 1
                cfh = sb.tile([P, nh], f32, tag=f"cfh{half}")
                nc.vector.tensor_scalar(cfh[:], coef[:, hs], 0.999, 0.5, op0=OP.min, op1=OP.mult)
                nc.vector.tensor_add(out=enc[:, hs], in0=tk1[:, hs], in1=cfh[:])
                nc.vector.tensor_mul(out=enc[:, hs], in0=enc[:, hs], in1=selm[:, hs])
                nc.vector.tensor_scalar_sub(enc[:, hs], enc[:, hs], 1.0)

            # PE warm-keeper while the top-2 DVE chain runs
            for wi in range(9):
                wps = psg.tile([P, P], f32, tag="mm2", name=f"warmC{wi}")
                nc.tensor.matmul(out=wps[:], lhsT=l_all[:, 0:P], rhs=l_all[:, 0:P],
                                 start=True, stop=True)

            # ---- D. compaction ----
            enc_t = sb.tile([NT, P], f32, tag="enc_t")
            tp1 = psg.tile([NT, P], f32, tag="mm3", name="enctp")
            nc.tensor.transpose(out=tp1[:], in_=enc[:], identity=idt[:])
            nc.vector.tensor_copy(out=enc_t[:], in_=tp1[:])

            sg_v = sb.tile([16, P], f32, tag="sgv")
            nf1 = sb.tile([1, 1], u32, tag="nf1")
            nc.gpsimd.sparse_gather(out=sg_v[:], in_=enc_t[:], num_found=nf1[:])

            # PE warm burst #1: fp32 MMs on enc_t while sparse_gather and the
            # compaction relayout run (keeps HAM at full clock into the GEMM)
            for wi in range(16):
                wps = psg.tile([P, P], f32, tag="mm2", name=f"warmA{wi}")
                nc.tensor.matmul(
                    out=wps[:], lhsT=enc_t[:, 0:P], rhs=enc_t[:, 0:P],
                    start=True, stop=True,
                )

            # valid-slot mask (sparse_gather tail is garbage on HW)
            nf_f = sb.tile([1, 1], f32, tag="nff")
            nc.vector.tensor_copy(out=nf_f[:], in_=nf1[:])
            ones16 = sb.tile([1, 16], f32, tag="ones16")
            nc.vector.memset(ones16[:], 1.0)
            nf_b_ps = psg.tile([16, 1], f32, tag="mm3", name="nfb")
            nc.tensor.matmul(out=nf_b_ps[:], lhsT=ones16[:], rhs=nf_f[:], start=True, stop=True)
            nf_b = sb.tile([16, 1], f32, tag="nfbs")
            nc.vector.tensor_copy(out=nf_b[:], in_=nf_b_ps[:])
            slot_mask = sb.tile([16, cf], u8, tag="slotm")
            nc.vector.tensor_tensor(
                out=slot_mask[:], in0=slotg[:],
                in1=nf_b[:].to_broadcast([16, cf]), op=OP.is_lt,
            )
            v_f = sb.tile([16, cf], f32, tag="vf")
            nc.vector.memset(v_f[:], 0.0)
            nc.vector.copy_predicated(out=v_f[:], mask=slot_mask[:], data=sg_v[:, 0:cf])

            # ---- D2. consume compacted slots in sparse_gather's native order.
            # Output column c = 128k + 8p + fl holds sg-slot 16(8k+fl) + p;
            # o_idx / cf_row / gathers all use the same (k, p, fl) order, so
            # no relayout transposes are needed. Invalid slots are exactly 0
            # (token 0, coef 0) and contribute nothing.
            # v_nf[k, 8p+fl] = v_f[p, 8k+fl]  (output-column order, row-major)
            # all relayout DMAs on the scalar ring: the sync ring is busy
            # streaming the gated w1/w3 prefetch right after the x stream
            v_nf = sb.tile([ntile, P], f32, tag="vnf")
            if rem:
                nc.vector.memset(v_nf[:], 0.0)
                nc.scalar.dma_start(out=v_nf[nft:ntile, 0:rem],
                                    in_=v_f[:, cfm:cf])
            for k in range(nft):
                nc.scalar.dma_start(out=v_nf[k:k + 1, :],
                                    in_=v_f[:, 8 * k:8 * k + 8])

            # packed row [1, cap] = v_nf flattened
            pkrow = sb.tile([1, cap], f32, tag="pkrow")
            nc.scalar.dma_start(out=pkrow[:, 0:nft * P], in_=v_nf[0:nft, :])
            if rem:
                nc.scalar.dma_start(out=pkrow[:, nft * P:cap],
                                    in_=v_nf[nft:ntile, 0:rem])

            # gather offsets [128, ntile] via one PE transpose
            vnf_ps = psg.tile([P, ntile], f32, tag="mm3", name="vnftp")
            nc.tensor.transpose(out=vnf_ps[:], in_=v_nf[:],
                                identity=idt[0:ntile, 0:ntile])
            idx_f = sb.tile([P, ntile], f32, tag="idxf")
            nc.vector.tensor_copy(out=idx_f[:], in_=vnf_ps[:])
            idx_sb = sb.tile([P, ntile], i32, tag="idxsb")
            nc.vector.tensor_copy(out=idx_sb[:], in_=idx_f[:])

            # PE warm bridge while gathers are issued
            for wi in range(8):
                wps = psg.tile([ntile, P], f32, tag="mm2", name=f"warmD{wi}")
                nc.tensor.matmul(
                    out=wps[:], lhsT=idx_f[:, 0:ntile], rhs=l_all[:, 0:P],
                    start=True, stop=True,
                )

            # coef row: frac(pkrow) * 2
            pk_i = sb.tile([1, cap], i32, tag="pki")
            nc.vector.tensor_copy(out=pk_i[:], in_=pkrow[:])
            pk_f = sb.tile([1, cap], f32, tag="pkf")
            nc.vector.tensor_copy(out=pk_f[:], in_=pk_i[:])
            cf_row = sb.tile([1, cap], f32, tag="cfrow")
            nc.vector.tensor_sub(out=cf_row[:], in0=pkrow[:], in1=pk_f[:])
            nc.vector.tensor_scalar_mul(cf_row[:], cf_row[:], 2.0)

            # PE warm burst #2: runs while gather tile 0 is in flight
            # (depends on v_nf, which lands before pkrow/cf_row)
            for wi in range(8):
                wps = psg.tile([P, P], f32, tag="mm2", name=f"warmB{wi}")
                nc.tensor.matmul(
                    out=wps[:], lhsT=v_nf[0:ntile, 0:P], rhs=v_nf[0:ntile, 0:P],
                    start=True, stop=True,
                )

            # ---- E. gather selected token rows; transposes are interleaved
            # with the first GEMM groups below ----
            xgT = [sb.tile([P, cap], bf16, tag=f"xgT{hc}", name=f"xgT{hc}") for hc in range(HC)]
            gtiles = [(k * P, P) for k in range(nft)] + ([(nft * P, rem)] if rem else [])
            gather_insts = []
            xg_tiles = []
            for gi, (goff, gn) in enumerate(gtiles):
                off_ap = idx_sb[0:gn, gi:gi + 1]
                xg = sbw.tile([gn, H], bf16, tag="xg", name=f"xg{gi}", bufs=5)
                xg_tiles.append(xg)
                gather_insts.append(nc.gpsimd.indirect_dma_start(
                    out=xg[:], out_offset=None,
                    in_=x[:],
                    in_offset=bass.IndirectOffsetOnAxis(ap=off_ap, axis=0),
                ))

            def emit_xpose(gi):
                goff, gn = gtiles[gi]
                xg = xg_tiles[gi]
                for hc in range(HC):
                    tpx = psg.tile([P, gn], bf16, tag="mm3", name=f"xtr{gi}{hc}")
                    nc.tensor.transpose(
                        out=tpx[:], in_=xg[:, hc * P:(hc + 1) * P],
                        identity=idtb[0:gn, 0:gn],
                    )
                    nc.vector.tensor_copy(
                        out=xgT[hc][:, goff:goff + gn], in_=tpx[:]
                    )

            # group 0 (cols 0:256) needs tiles 0-1 only
            emit_xpose(0)
            emit_xpose(1)

            # ---- F. coef broadcast [128, cap] (bf16 matmul: 1-pass) ----
            onesP = sb.tile([1, P], bf16, tag="onesP")
            nc.vector.memset(onesP[:], 1.0)
            cf_row_b = sb.tile([1, cap], bf16, tag="cfrowb")
            nc.vector.tensor_copy(out=cf_row_b[:], in_=cf_row[:])
            cbc = sb.tile([P, cap], f32, tag="cbc")
            for goff, gn in groups:
                cb_ps = psg.tile([P, gn], f32, tag="mm2", name=f"cb{goff}")
                nc.tensor.matmul(
                    out=cb_ps[:], lhsT=onesP[:],
                    rhs=cf_row_b[:, goff:goff + gn], start=True, stop=True,
                )
                nc.vector.tensor_copy(out=cbc[:, goff:goff + gn], in_=cb_ps[:])

            # ---- G. h1 = x@w1, h3 = x@w3 (transposed), fused silu*mul ----
            last_x = x_dma_insts[-1].ins
            last_x2 = x_dma_insts[-2].ins
            w_gate = []    # gated behind the x stream
            w_gate_b = []  # gated behind the token gathers

            actT = [sb.tile([P, cap], bf16, tag=f"actT{ic}", name=f"actT{ic}") for ic in range(IC)]
            w1_slices = {}
            w1_sl = w3_sl = None

            def gemm13(ic, goff, gn):
                w1s_t, w3s_t = w1_slices[ic // 4]
                iq = (ic % 4) * P
                gs = slice(goff, goff + gn)
                ps1 = psg.tile([P, gn], f32, tag="mm0", name=f"ps1_{ic}_{goff}")
                ps3 = psg.tile([P, gn], f32, tag="mm1", name=f"ps3_{ic}_{goff}")
                for hc in range(HC):
                    nc.tensor.matmul(
                        out=ps1[:],
                        lhsT=w1s_t[:, hc * 512 + iq:hc * 512 + iq + P],
                        rhs=xgT[hc][:, gs],
                        start=(hc == 0), stop=(hc == HC - 1),
                    )
                for hc in range(HC):
                    nc.tensor.matmul(
                        out=ps3[:],
                        lhsT=w3s_t[:, hc * 512 + iq:hc * 512 + iq + P],
                        rhs=xgT[hc][:, gs],
                        start=(hc == 0), stop=(hc == HC - 1),
                    )
                sl = sbw.tile([P, gn], f32, tag="silu", name=f"sl{ic}_{goff}")
                nc.scalar.activation(out=sl[:], in_=ps1[:], func=AF.Silu)
                nc.vector.tensor_mul(out=actT[ic][:, gs], in0=sl[:], in1=ps3[:])

            pend_g1 = []
            for ic in range(IC):
                if ic % 4 == 0:
                    icg = ic // 4
                    # blocked load: 8KB contiguous lines per partition
                    w1_sl = sbw.tile([P, 4096], bf16, tag="w1sl", bufs=3)
                    iw1 = nc.sync.dma_start(
                        out=w1_sl[:], in_=w1[icg * P:(icg + 1) * P, :])
                    w3_sl = sbw.tile([P, 4096], bf16, tag="w3sl", bufs=3)
                    iw3 = nc.sync.dma_start(
                        out=w3_sl[:], in_=w3[icg * P:(icg + 1) * P, :])
                    w1_slices[icg] = (w1_sl, w3_sl)
                    if icg < 2:
                        w_gate += [iw1, iw3]
                    elif icg == 2:
                        w_gate_b += [iw1, iw3]
                gemm13(ic, *groups[0])
                if ic < len(gtiles) - 2:
                    # transpose one more gather tile between early GEMM groups
                    emit_xpose(2 + ic)
                    pend_g1.append(ic)
                else:
                    for j in pend_g1:
                        gemm13(j, *groups[1])
                    pend_g1 = []
                    gemm13(ic, *groups[1])

            # ---- H. yT = (act @ w2).T * coef ----
            for hc in range(HC):
                w2_sl = sbw.tile([P, II], bf16, tag="w2sl", bufs=3)
                iw2 = nc.sync.dma_start(
                    out=w2_sl[:], in_=w2[hc * P:(hc + 1) * P, :])
                if hc < 3:
                    w_gate_b.append(iw2)
                for goff, gn in groups:
                    gs = slice(goff, goff + gn)
                    pso = psg.tile([P, gn], f32, tag="mm2", name=f"pso{hc}_{goff}")
                    for ic in range(IC):
                        nc.tensor.matmul(
                            out=pso[:],
                            lhsT=w2_sl[:, ic * P:(ic + 1) * P],
                            rhs=actT[ic][:, gs],
                            start=(ic == 0), stop=(ic == IC - 1),
                        )
                    yt_sb = sbw.tile([P, gn], bf16, tag="yt", name=f"yt{hc}_{goff}")
                    nc.vector.tensor_mul(out=yt_sb[:], in0=pso[:], in1=cbc[:, gs])
                    nc.sync.dma_start(
                        out=o_yt[hc * P:(hc + 1) * P, gs], in_=yt_sb[:]
                    )

            # ---- outputs that are off the critical path ----
            nc.scalar.dma_start(out=o_cnt[:], in_=nf1[:])
            nc.scalar.dma_start(out=o_idx[:], in_=pk_i[:])

            # gate the leading weight loads behind the x-transpose stream;
            # the rest behind the token gathers (so gathers get HBM bandwidth)
            for wi_inst in w_gate:
                add_dep_helper(wi_inst.ins, last_x, sync=True,
                               reason="weight stream after router x stream")
                add_dep_helper(wi_inst.ins, last_x2, sync=True,
                               reason="weight stream after router x stream")
            last_gather = gather_insts[-1].ins
            for wi_inst in w_gate_b:
                add_dep_helper(wi_inst.ins, last_gather, sync=True,
                               reason="bulk weight stream after token gathers")

    nc.compile()
    return nc


def _get_built(cap):
    if cap not in _build_cache:
        _build_cache[cap] = _build(cap)
    return _build_cache[cap]


def _run(cap, hs, gate_w, w1s, w2s, w3s, trace=False):
    import ml_dtypes
    from concourse.bass_utils import run_bass_kernel_spmd

    nc = _get_built(cap)

    bf = ml_dtypes.bfloat16
    f8 = ml_dtypes.float8_e4m3
    x_hi = hs.astype(bf)
    x_lo = hs - x_hi.astype(np.float32)
    xth_np = np.ascontiguousarray(x_hi.T)
    xtl8_np = np.ascontiguousarray((x_lo * 2.0 ** 7).astype(f8).T)
    gw8_np = (gate_w * 2.0 ** 4).astype(f8)

    def block_w13(w):
        # [hc*128+p, icg*512+q] -> [icg*128+p, hc*512+q]
        return np.ascontiguousarray(
            w.astype(bf).reshape(8, 128, 8, 512).transpose(2, 1, 0, 3).reshape(1024, 4096))

    def block_w2(w):
        # [ic*128+p, hc*128+h] -> [hc*128+p, ic*128+h]
        return np.ascontiguousarray(
            w.astype(bf).reshape(32, 128, 8, 128).transpose(2, 1, 0, 3).reshape(1024, 4096))
    gw_hi = gate_w.astype(bf)
    gw_lo = (gate_w - gw_hi.astype(np.float32)).astype(bf)
    x_bf = np.ascontiguousarray(x_hi)
    oh_base = np.zeros((P, NT, E), np.float32)
    tokid1_np = (np.arange(NT)[None, :] * P + np.arange(P)[:, None] + 1.0).astype(np.float32)
    slotg_np = (np.arange(cap // 16)[None, :] * 16 + np.arange(16)[:, None]).astype(np.float32)
    ident_np = np.eye(P, dtype=np.float32)

    in_maps = []
    for c in range(NCORES):
        oh_c = oh_base.copy()
        oh_c[:, :, c] = 1.0
        in_maps.append({
            "xth": xth_np,
            "xtl8": xtl8_np,
            "x": x_bf,
            "gwh": gw_hi,
            "gwl": gw_lo,
            "gw8": gw8_np,
            "w1": block_w13(w1s[c]),
            "w3": block_w13(w3s[c]),
            "w2": block_w2(w2s[c]),
            "oh": oh_c.reshape(P, NT * E),
            "tokid1": tokid1_np,
            "slotg": slotg_np,
            "ident": ident_np,
        })

    res = run_bass_kernel_spmd(nc, in_maps, list(range(NCORES)), trace=trace)
    return res


def kernel(hidden_states, gate_w, w1s, w2s, w3s, _trace=False, _cap=560):
    hs = np.ascontiguousarray(np.asarray(hidden_states, dtype=np.float32))
    gate_w = np.ascontiguousarray(np.asarray(gate_w, dtype=np.float32))
    w1s = np.asarray(w1s, dtype=np.float32)
    w2s = np.asarray(w2s, dtype=np.float32)
    w3s = np.asarray(w3s, dtype=np.float32)

    cap = _cap
    while True:
        res = _run(cap, hs, gate_w, w1s, w2s, w3s, trace=_trace)
        counts = [int(res.results[c]["o_cnt"].ravel()[0]) for c in range(NCORES)]
        if max(counts) <= cap:
            break
        # capacity overflow (won't happen for sane routing): rebuild bigger
        cap = 2048 if max(counts) > 1024 else 1024

    out = np.zeros((T, H), dtype=np.float32)
    for c in range(NCORES):
        r = res.results[c]
        # columns are in sparse_gather-native order; invalid slots are
        # exactly zero (token 0, coef 0), so add.at over all columns is safe
        idx = np.asarray(r["o_idx"])
        y = np.ascontiguousarray(r["o_yt"].T).astype(np.float32)
        np.add.at(out, idx, y)
    kernel._last_results = res
    return out


# revision 43
# speedup vs baseline: 1.0065x; 1.0065x over previous
"""MoE (8 experts, top-2) Trainium2 kernel, expert-parallel across 8 NeuronCores.

Strategy:
  - Each core owns one expert (weights sharded along the expert axis; gate
    replicated). Everything data-dependent runs on device:
      * router logits (bf16 hi/lo split, fp32 accuracy), token-major streamed
        in 2 halves so top-2 for half 0 overlaps the stream of half 1
      * top-2 + renormalized gate weights (DVE), per-expert token compaction
        (gpsimd sparse_gather)
      * compacted (tokid, coef) relayout via one PE transpose + SBUF->SBUF
        DMAs with contiguous runs (no element-granular DRAM round trip)
      * token dispatch (indirect DMA gather of selected token rows) + PE
        transposes to [H, cap]
      * expert MLP GEMMs in bf16: (silu(x@w1) * (x@w3)) @ w2, scaled by the
        gate coefficient
  - Weight streams are gated behind the x-transpose stream so the router
    phase runs at full HBM bandwidth.
  - Each core returns its expert's (transposed, bf16) token outputs + the
    compacted token index list; the host scatter-adds the 8 partial outputs.
"""
import sys

sys.path.insert(0, "/opt/trn_rl_repo")

import numpy as np

T, H, II, E = 2048, 1024, 4096, 8
P = 128
NT = T // P          # 16 token tiles
HC = H // P          # 8 hidden chunks
IC = II // P         # 32 intermediate chunks
NCORES = 8
NHALF = 2            # router stream halves
TH = T // NHALF      # tokens per half
NTH = TH // P        # token tiles per half


def _groups(cap):
    """Split cap token columns into PSUM-sized GEMM groups (<=512 each).
    First group is 256 (= 2 gather tiles) so it can start earliest."""
    gs, off = [], 0
    first = min(256, cap)
    gs.append((0, first))
    off = first
    while off < cap:
        n = min(512, cap - off)
        gs.append((off, n))
        off += n
    return gs


_build_cache = {}


def _build(cap):
    """Build + schedule the per-core Tile kernel for token capacity `cap`."""
    import concourse.bass as bass
    import concourse.bacc as bacc
    import concourse.mybir as mybir
    from concourse.tile import TileContext
    from concourse.tile_rust import add_dep_helper

    f32 = mybir.dt.float32
    i32 = mybir.dt.int32
    u32 = mybir.dt.uint32
    u8 = mybir.dt.uint8
    bf16 = mybir.dt.bfloat16
    AF = mybir.ActivationFunctionType
    OP = mybir.AluOpType

    assert cap % 16 == 0
    cf = cap // 16       # columns of [16, cf] compacted layout
    nft = cap // P       # full 128-token gather tiles
    rem = cap - nft * P  # remainder gather tile rows
    ntile = nft + (1 if rem else 0)
    groups = _groups(cap)
    cfm = (cf // 8) * 8  # vt rows covered by full gather tiles * 8

    nc = bacc.Bacc("TRN2", target_bir_lowering=False)

    # ---- I/O ----
    f8 = mybir.dt.float8e4
    xth = nc.declare_dram_parameter("xth", [H, T], bf16, isOutput=False)
    # low-order residual of x, scaled by 2^7, in fp8 (verified: 0 top-2 flips)
    xtl8 = nc.declare_dram_parameter("xtl8", [H, T], f8, isOutput=False)
    x = nc.declare_dram_parameter("x", [T, H], bf16, isOutput=False)
    gwh = nc.declare_dram_parameter("gwh", [H, E], bf16, isOutput=False)
    gwl = nc.declare_dram_parameter("gwl", [H, E], bf16, isOutput=False)
    gw8d = nc.declare_dram_parameter("gw8", [H, E], f8, isOutput=False)  # 16*g in fp8
    # w1/w3 host-blocked: [icg*128+p, hc*512+q] = w1[hc*128+p, (icg*4+q//128)*128+q%128]
    # -> one [128, 4096] DMA per icg with 8KB contiguous per-partition lines
    w1 = nc.declare_dram_parameter("w1", [1024, 4096], bf16, isOutput=False)
    w3 = nc.declare_dram_parameter("w3", [1024, 4096], bf16, isOutput=False)
    # w2 host-blocked: [hc*128+p, ic*128+h] = w2[ic*128+p, hc*128+h]
    w2 = nc.declare_dram_parameter("w2", [1024, 4096], bf16, isOutput=False)
    oh = nc.declare_dram_parameter("oh", [P, NT * E], f32, isOutput=False)
    tokid1 = nc.declare_dram_parameter("tokid1", [P, NT], f32, isOutput=False)
    slotg_d = nc.declare_dram_parameter("slotg", [16, cf], f32, isOutput=False)
    ident = nc.declare_dram_parameter("ident", [P, P], f32, isOutput=False)

    o_yt = nc.declare_dram_parameter("o_yt", [H, cap], bf16, isOutput=True)
    o_idx = nc.declare_dram_parameter("o_idx", [cap], i32, isOutput=True)
    o_cnt = nc.declare_dram_parameter("o_cnt", [1, 1], u32, isOutput=True)

    with TileContext(nc) as tc:
        with (
            tc.tile_pool(name="sb", bufs=1) as sb,
            tc.tile_pool(name="sbw", bufs=2) as sbw,
            tc.tile_pool(name="psum", bufs=2, space="PSUM") as psg,
        ):
            # ---- constants (scalar HWDGE ring; small) ----
            idt = sb.tile([P, P], f32, tag="idt")
            nc.scalar.dma_start(out=idt[:], in_=ident[:])
            idtb = sb.tile([P, P], bf16, tag="idtb")
            nc.vector.tensor_copy(out=idtb[:], in_=idt[:])
            gw_h = sb.tile([P, HC * E], bf16, tag="gwh")
            nc.scalar.dma_start(
                out=gw_h[:].rearrange("p (hc e) -> p hc e", e=E),
                in_=gwh[:].rearrange("(hc p) e -> p hc e", p=P),
            )
            gw_l = sb.tile([P, HC * E], bf16, tag="gwl")
            nc.scalar.dma_start(
                out=gw_l[:].rearrange("p (hc e) -> p hc e", e=E),
                in_=gwl[:].rearrange("(hc p) e -> p hc e", p=P),
            )
            gw_8 = sb.tile([P, HC * E], f8, tag="gw8")
            nc.scalar.dma_start(
                out=gw_8[:].rearrange("p (hc e) -> p hc e", e=E),
                in_=gw8d[:].rearrange("(hc p) e -> p hc e", p=P),
            )

            # PE warm-up: flip HAM to full clock before the first router MMs
            scr = sb.tile([P, 512], bf16, tag="scr")
            nc.vector.memset(scr[:], 0.0)
            for wi in range(12):
                wps = psg.tile([P, 512], f32, tag="mm2", name=f"warm0_{wi}")
                nc.tensor.matmul(out=wps[:], lhsT=idtb[:], rhs=scr[:],
                                 start=True, stop=True)
            # ---- A. router: logitsT [8, 2048], token-major in 2 halves ----
            logitsT = sb.tile([E, T], f32, tag="logitsT")
            l_all = sb.tile([P, NT * E], f32, tag="l_all")
            l3 = l_all[:].rearrange("p (t e) -> p t e", e=E)

            # per-half top-2 intermediates
            m1 = sb.tile([P, NT, 1], f32, tag="m1")
            m2 = sb.tile([P, NT, 1], f32, tag="m2")
            wt1 = sb.tile([P, NT], f32, tag="wt1")
            wt2 = sb.tile([P, NT], f32, tag="wt2")
            coef = sb.tile([P, NT], f32, tag="coef")
            selm = sb.tile([P, NT], f32, tag="selm")
            enc = sb.tile([P, NT], f32, tag="enc")

            # big consts via the (idle) gpsimd SWDGE ring — keeps the two
            # HWDGE rings free for the x stream
            oh_sb = sb.tile([P, NT * E], f32, tag="oh")
            nc.gpsimd.dma_start(out=oh_sb[:], in_=oh[:])
            tk1 = sb.tile([P, NT], f32, tag="tk1")
            nc.gpsimd.dma_start(out=tk1[:], in_=tokid1[:])
            slotg = sb.tile([16, cf], f32, tag="slotg")
            nc.gpsimd.dma_start(out=slotg[:], in_=slotg_d[:])

            x_dma_insts = []
            for half in range(NHALF):
                t0 = half * TH
                ts = slice(t0, t0 + TH)
                nq = TH // 512  # 512-col psum groups in this half
                ps_l = [
                    psg.tile([E, 512], f32, tag=f"mm{q}", name=f"psl{half}{q}")
                    for q in range(nq)
                ]
                ps_8 = [
                    psg.tile([E, 512], f32, tag=f"mm{q + 2}", name=f"ps8{half}{q}")
                    for q in range(nq)
                ]
                for c4 in range(HC // 4):
                    # 4 hidden chunks per DMA: 1MB hi (bf16) + 0.5MB lo (fp8);
                    # cross-assign rings so each carries ~1.5MB per half
                    e_h = nc.sync if c4 == 0 else nc.scalar
                    e_l = nc.scalar if c4 == 0 else nc.sync
                    xt_h = sbw.tile([P, 4 * TH], bf16, tag="xth", bufs=2)
                    ih = e_h.dma_start(
                        out=xt_h[:].rearrange("p (c t) -> p c t", c=4),
                        in_=xth[c4 * 4 * P:(c4 + 1) * 4 * P, ts].rearrange(
                            "(c p) t -> p c t", p=P),
                    )
                    xt_l = sbw.tile([P, 4 * TH], f8, tag="xtl", bufs=2)
                    il = e_l.dma_start(
                        out=xt_l[:].rearrange("p (c t) -> p c t", c=4),
                        in_=xtl8[c4 * 4 * P:(c4 + 1) * 4 * P, ts].rearrange(
                            "(c p) t -> p c t", p=P),
                    )
                    x_dma_insts.append(ih)
                    x_dma_insts.append(il)
                    for ci in range(4):
                        hc = 4 * c4 + ci
                        for q in range(nq):
                            qs = slice(ci * TH + q * 512, ci * TH + (q + 1) * 512)
                            st = (hc == 0)
                            sp = (hc == HC - 1)
                            nc.tensor.matmul(
                                out=ps_l[q][:],
                                lhsT=gw_h[:, hc * E:(hc + 1) * E],
                                rhs=xt_h[:, qs],
                                start=st, stop=False,
                            )
                            nc.tensor.matmul(
                                out=ps_l[q][:],
                                lhsT=gw_l[:, hc * E:(hc + 1) * E],
                                rhs=xt_h[:, qs],
                                start=False, stop=sp,
                            )
                            nc.tensor.matmul(
                                out=ps_8[q][:],
                                lhsT=gw_8[:, hc * E:(hc + 1) * E],
                                rhs=xt_l[:, qs],
                                start=st, stop=sp,
                            )
                for q in range(nq):
                    # logits = ps_l + 2^-11 * ps_8  (fp8 term carries 2^11 scale)
                    t8 = sbw.tile([E, 512], f32, tag="t8")
                    nc.scalar.activation(out=t8[:], in_=ps_8[q][:],
                                         func=AF.Copy, scale=2.0 ** -11)
                    nc.vector.tensor_add(
                        out=logitsT[:, t0 + q * 512:t0 + (q + 1) * 512],
                        in0=ps_l[q][:], in1=t8[:],
                    )

                # ---- B. transpose this half's logits into l_all ----
                for ci in range(half * NTH, (half + 1) * NTH):
                    tp = psg.tile([P, E], f32, tag="mm3", name=f"ltp{ci}")
                    nc.tensor.transpose(
                        out=tp[:],
                        in_=logitsT[:, ci * P:(ci + 1) * P],
                        identity=idt[0:E, 0:E],
                    )
                    nc.vector.tensor_copy(out=l_all[:, ci * E:(ci + 1) * E], in_=tp[:])

                # ---- C. top-2 + coef for this half ----
                hs = slice(half * NTH, (half + 1) * NTH)
                l3h = l3[:, hs, :]
                nh = NTH
                m1a = sb.tile([P, nh, 4], f32, tag=f"m1a{half}")
                m2a = sb.tile([P, nh, 4], f32, tag=f"m2a{half}")
                nc.vector.tensor_tensor(out=m1a[:], in0=l3h[:, :, 0::2], in1=l3h[:, :, 1::2], op=OP.max)
                nc.vector.tensor_tensor(out=m2a[:], in0=l3h[:, :, 0::2], in1=l3h[:, :, 1::2], op=OP.min)
                m1b = sb.tile([P, nh, 2], f32, tag=f"m1b{half}")
                m2b = sb.tile([P, nh, 2], f32, tag=f"m2b{half}")
                tmin = sb.tile([P, nh, 2], f32, tag=f"tmin{half}")
                nc.vector.tensor_tensor(out=m1b[:], in0=m1a[:, :, 0::2], in1=m1a[:, :, 1::2], op=OP.max)
                nc.vector.tensor_tensor(out=tmin[:], in0=m1a[:, :, 0::2], in1=m1a[:, :, 1::2], op=OP.min)
                nc.vector.tensor_tensor(out=m2b[:], in0=m2a[:, :, 0::2], in1=m2a[:, :, 1::2], op=OP.max)
                nc.vector.tensor_tensor(out=m2b[:], in0=m2b[:], in1=tmin[:], op=OP.max)
                m1h = m1[:, hs, :]
                m2h = m2[:, hs, :]
                tmin2 = sb.tile([P, nh, 1], f32, tag=f"tmin2{half}")
                nc.vector.tensor_tensor(out=m1h, in0=m1b[:, :, 0:1], in1=m1b[:, :, 1:2], op=OP.max)
                nc.vector.tensor_tensor(out=tmin2[:], in0=m1b[:, :, 0:1], in1=m1b[:, :, 1:2], op=OP.min)
                nc.vector.tensor_tensor(out=m2h, in0=m2b[:, :, 0:1], in1=m2b[:, :, 1:2], op=OP.max)
                nc.vector.tensor_tensor(out=m2h, in0=m2h, in1=tmin2[:], op=OP.max)

                dq = sb.tile([P, nh], f32, tag=f"dq{half}")
                nc.vector.tensor_sub(out=dq[:], in0=m2[:, hs, 0], in1=m1[:, hs, 0])
                q_t = sb.tile([P, nh], f32, tag=f"q{half}")
                nc.scalar.activation(out=q_t[:], in_=dq[:], func=AF.Exp)
                s_t = sb.tile([P, nh], f32, tag=f"s{half}")
                nc.vector.tensor_scalar_add(s_t[:], q_t[:], 1.0)
                nc.vector.reciprocal(wt1[:, hs], s_t[:])
                nc.vector.tensor_mul(out=wt2[:, hs], in0=q_t[:], in1=wt1[:, hs])

                le_m = sb.tile([P, nh, E], f32, tag=f"lem{half}")
                nc.vector.tensor_mul(
                    out=le_m[:], in0=l3h,
                    in1=oh_sb[:].rearrange("p (t e) -> p t e", e=E)[:, hs, :],
                )
                le = sb.tile([P, nh], f32, tag=f"le{half}")
                nc.vector.reduce_sum(
                    out=le[:].rearrange("p (t o) -> p t o", o=1),
                    in_=le_m[:],
                    axis=mybir.AxisListType.X,
                )
                eq1 = sb.tile([P, nh], f32, tag=f"eq1{half}")
                eq2 = sb.tile([P, nh], f32, tag=f"eq2{half}")
                nc.vector.tensor_tensor(out=eq1[:], in0=le[:], in1=m1[:, hs, 0], op=OP.is_equal)
                nc.vector.tensor_tensor(out=eq2[:], in0=le[:], in1=m2[:, hs, 0], op=OP.is_equal)
                t1 = sb.tile([P, nh], f32, tag=f"t1{half}")
                nc.vector.tensor_mul(out=coef[:, hs], in0=eq1[:], in1=wt1[:, hs])
                nc.vector.tensor_mul(out=t1[:], in0=eq2[:], in1=wt2[:, hs])
                nc.vector.tensor_add(out=coef[:, hs], in0=coef[:, hs], in1=t1[:])
                nc.vector.tensor_add(out=selm[:, hs], in0=eq1[:], in1=eq2[:])

                # enc = (tokid + 1 + min(coef,.999)/2) * selm - 1
                cfh = sb.tile([P, nh], f32, tag=f"cfh{half}")
                nc.vector.tensor_scalar(cfh[:], coef[:, hs], 0.999, 0.5, op0=OP.min, op1=OP.mult)
                nc.vector.tensor_add(out=enc[:, hs], in0=tk1[:, hs], in1=cfh[:])
                nc.vector.tensor_mul(out=enc[:, hs], in0=enc[:, hs], in1=selm[:, hs])
                nc.vector.tensor_scalar_sub(enc[:, hs], enc[:, hs], 1.0)

            # PE warm-keeper while the top-2 DVE chain runs
            for wi in range(9):
                wps = psg.tile([P, P], f32, tag="mm2", name=f"warmC{wi}")
                nc.tensor.matmul(out=wps[:], lhsT=l_all[:, 0:P], rhs=l_all[:, 0:P],
                                 start=True, stop=True)

            # ---- D. compaction ----
            enc_t = sb.tile([NT, P], f32, tag="enc_t")
            tp1 = psg.tile([NT, P], f32, tag="mm3", name="enctp")
            nc.tensor.transpose(out=tp1[:], in_=enc[:], identity=idt[:])
            nc.vector.tensor_copy(out=enc_t[:], in_=tp1[:])

            sg_v = sb.tile([16, P], f32, tag="sgv")
            nf1 = sb.tile([1, 1], u32, tag="nf1")
            nc.gpsimd.sparse_gather(out=sg_v[:], in_=enc_t[:], num_found=nf1[:])

            # PE warm burst #1: fp32 MMs on enc_t while sparse_gather and the
            # compaction relayout run (keeps HAM at full clock into the GEMM)
            for wi in range(16):
                wps = psg.tile([P, P], f32, tag="mm2", name=f"warmA{wi}")
                nc.tensor.matmul(
                    out=wps[:], lhsT=enc_t[:, 0:P], rhs=enc_t[:, 0:P],
                    start=True, stop=True,
                )

            # valid-slot mask (sparse_gather tail is garbage on HW)
            nf_f = sb.tile([1, 1], f32, tag="nff")
            nc.vector.tensor_copy(out=nf_f[:], in_=nf1[:])
            ones16 = sb.tile([1, 16], f32, tag="ones16")
            nc.vector.memset(ones16[:], 1.0)
            nf_b_ps = psg.tile([16, 1], f32, tag="mm3", name="nfb")
            nc.tensor.matmul(out=nf_b_ps[:], lhsT=ones16[:], rhs=nf_f[:], start=True, stop=True)
            nf_b = sb.tile([16, 1], f32, tag="nfbs")
            nc.vector.tensor_copy(out=nf_b[:], in_=nf_b_ps[:])
            slot_mask = sb.tile([16, cf], u8, tag="slotm")
            nc.vector.tensor_tensor(
                out=slot_mask[:], in0=slotg[:],
                in1=nf_b[:].to_broadcast([16, cf]), op=OP.is_lt,
            )
            v_f = sb.tile([16, cf], f32, tag="vf")
            nc.vector.memset(v_f[:], 0.0)
            nc.vector.copy_predicated(out=v_f[:], mask=slot_mask[:], data=sg_v[:, 0:cf])

            # ---- D2. consume compacted slots in sparse_gather's native order.
            # Output column c = 128k + 8p + fl holds sg-slot 16(8k+fl) + p;
            # o_idx / cf_row / gathers all use the same (k, p, fl) order, so
            # no relayout transposes are needed. Invalid slots are exactly 0
            # (token 0, coef 0) and contribute nothing.
            # v_nf[k, 8p+fl] = v_f[p, 8k+fl]  (output-column order, row-major)
            # all relayout DMAs on the scalar ring: the sync ring is busy
            # streaming the gated w1/w3 prefetch right after the x stream
            v_nf = sb.tile([ntile, P], f32, tag="vnf")
            if rem:
                nc.vector.memset(v_nf[:], 0.0)
                nc.scalar.dma_start(out=v_nf[nft:ntile, 0:rem],
                                    in_=v_f[:, cfm:cf])
            for k in range(nft):
                nc.scalar.dma_start(out=v_nf[k:k + 1, :],
                                    in_=v_f[:, 8 * k:8 * k + 8])

            # packed row [1, cap] = v_nf flattened
            pkrow = sb.tile([1, cap], f32, tag="pkrow")
            nc.scalar.dma_start(out=pkrow[:, 0:nft * P], in_=v_nf[0:nft, :])
            if rem:
                nc.scalar.dma_start(out=pkrow[:, nft * P:cap],
                                    in_=v_nf[nft:ntile, 0:rem])

            # gather offsets [128, ntile] via one PE transpose
            vnf_ps = psg.tile([P, ntile], f32, tag="mm3", name="vnftp")
            nc.tensor.transpose(out=vnf_ps[:], in_=v_nf[:],
                                identity=idt[0:ntile, 0:ntile])
            idx_f = sb.tile([P, ntile], f32, tag="idxf")
            nc.vector.tensor_copy(out=idx_f[:], in_=vnf_ps[:])
            idx_sb = sb.tile([P, ntile], i32, tag="idxsb")
            nc.vector.tensor_copy(out=idx_sb[:], in_=idx_f[:])

            # PE warm bridge while gathers are issued
            for wi in range(8):
                wps = psg.tile([ntile, P], f32, tag="mm2", name=f"warmD{wi}")
                nc.tensor.matmul(
                    out=wps[:], lhsT=idx_f[:, 0:ntile], rhs=l_all[:, 0:P],
                    start=True, stop=True,
                )

            # coef row: frac(pkrow) * 2
            pk_i = sb.tile([1, cap], i32, tag="pki")
            nc.vector.tensor_copy(out=pk_i[:], in_=pkrow[:])
            pk_f = sb.tile([1, cap], f32, tag="pkf")
            nc.vector.tensor_copy(out=pk_f[:], in_=pk_i[:])
            cf_row = sb.tile([1, cap], f32, tag="cfrow")
            nc.vector.tensor_sub(out=cf_row[:], in0=pkrow[:], in1=pk_f[:])
            nc.vector.tensor_scalar_mul(cf_row[:], cf_row[:], 2.0)

            # PE warm burst #2: runs while gather tile 0 is in flight
            # (depends on v_nf, which lands before pkrow/cf_row)
            for wi in range(8):
                wps = psg.tile([P, P], f32, tag="mm2", name=f"warmB{wi}")
                nc.tensor.matmul(
                    out=wps[:], lhsT=v_nf[0:ntile, 0:P], rhs=v_nf[0:ntile, 0:P],
                    start=True, stop=True,
                )

            # ---- E. gather selected token rows; transposes are interleaved
            # with the first GEMM groups below ----
            xgT = [sb.tile([P, cap], bf16, tag=f"xgT{hc}", name=f"xgT{hc}") for hc in range(HC)]
            gtiles = [(k * P, P) for k in range(nft)] + ([(nft * P, rem)] if rem else [])
            gather_insts = []
            xg_tiles = []
            for gi, (goff, gn) in enumerate(gtiles):
                off_ap = idx_sb[0:gn, gi:gi + 1]
                xg = sbw.tile([gn, H], bf16, tag="xg", name=f"xg{gi}", bufs=5)
                xg_tiles.append(xg)
                gather_insts.append(nc.gpsimd.indirect_dma_start(
                    out=xg[:], out_offset=None,
                    in_=x[:],
                    in_offset=bass.IndirectOffsetOnAxis(ap=off_ap, axis=0),
                ))

            def emit_xpose(gi):
                goff, gn = gtiles[gi]
                xg = xg_tiles[gi]
                for hc in range(HC):
                    tpx = psg.tile([P, gn], bf16, tag="mm3", name=f"xtr{gi}{hc}")
                    nc.tensor.transpose(
                        out=tpx[:], in_=xg[:, hc * P:(hc + 1) * P],
                        identity=idtb[0:gn, 0:gn],
                    )
                    nc.vector.tensor_copy(
                        out=xgT[hc][:, goff:goff + gn], in_=tpx[:]
                    )

            # group 0 (cols 0:256) needs tiles 0-1 only
            emit_xpose(0)
            emit_xpose(1)

            # ---- F. coef broadcast [128, cap] (bf16 matmul: 1-pass) ----
            onesP = sb.tile([1, P], bf16, tag="onesP")
            nc.vector.memset(onesP[:], 1.0)
            cf_row_b = sb.tile([1, cap], bf16, tag="cfrowb")
            nc.vector.tensor_copy(out=cf_row_b[:], in_=cf_row[:])
            cbc = sb.tile([P, cap], f32, tag="cbc")
            for goff, gn in groups:
                cb_ps = psg.tile([P, gn], f32, tag="mm2", name=f"cb{goff}")
                nc.tensor.matmul(
                    out=cb_ps[:], lhsT=onesP[:],
                    rhs=cf_row_b[:, goff:goff + gn], start=True, stop=True,
                )
                nc.vector.tensor_copy(out=cbc[:, goff:goff + gn], in_=cb_ps[:])

            # ---- G. h1 = x@w1, h3 = x@w3 (transposed), fused silu*mul ----
            last_x = x_dma_insts[-1].ins
            last_x2 = x_dma_insts[-2].ins
            w_gate = []    # gated behind the x stream
            w_gate_b = []  # gated behind the token gathers

            actT = [sb.tile([P, cap], bf16, tag=f"actT{ic}", name=f"actT{ic}") for ic in range(IC)]
            w1_slices = {}
            w1_sl = w3_sl = None

            def gemm13(ic, goff, gn):
                w1s_t, w3s_t = w1_slices[ic // 4]
                iq = (ic % 4) * P
                gs = slice(goff, goff + gn)
                ps1 = psg.tile([P, gn], f32, tag="mm0", name=f"ps1_{ic}_{goff}")
                ps3 = psg.tile([P, gn], f32, tag="mm1", name=f"ps3_{ic}_{goff}")
                for hc in range(HC):
                    nc.tensor.matmul(
                        out=ps1[:],
                        lhsT=w1s_t[:, hc * 512 + iq:hc * 512 + iq + P],
                        rhs=xgT[hc][:, gs],
                        start=(hc == 0), stop=(hc == HC - 1),
                    )
                for hc in range(HC):
                    nc.tensor.matmul(
                        out=ps3[:],
                        lhsT=w3s_t[:, hc * 512 + iq:hc * 512 + iq + P],
                        rhs=xgT[hc][:, gs],
                        start=(hc == 0), stop=(hc == HC - 1),
                    )
                sl = sbw.tile([P, gn], f32, tag="silu", name=f"sl{ic}_{goff}")
                nc.scalar.activation(out=sl[:], in_=ps1[:], func=AF.Silu)
                nc.vector.tensor_mul(out=actT[ic][:, gs], in0=sl[:], in1=ps3[:])

            pend_g1 = []
            for ic in range(IC):
                if ic % 4 == 0:
                    icg = ic // 4
                    # blocked load: 8KB contiguous lines per partition
                    w1_sl = sbw.tile([P, 4096], bf16, tag="w1sl", bufs=3)
                    iw1 = nc.sync.dma_start(
                        out=w1_sl[:], in_=w1[icg * P:(icg + 1) * P, :])
                    w3_sl = sbw.tile([P, 4096], bf16, tag="w3sl", bufs=3)
                    iw3 = nc.sync.dma_start(
                        out=w3_sl[:], in_=w3[icg * P:(icg + 1) * P, :])
                    w1_slices[icg] = (w1_sl, w3_sl)
                    if icg < 2:
                        w_gate += [iw1, iw3]
                    elif icg == 2:
                        w_gate_b += [iw1, iw3]
                gemm13(ic, *groups[0])
                if ic < len(gtiles) - 2:
                    # transpose one more gather tile between early GEMM groups
                    emit_xpose(2 + ic)
                    pend_g1.append(ic)
                else:
                    for j in pend_g1:
                        gemm13(j, *groups[1])
                    pend_g1 = []
                    gemm13(ic, *groups[1])

            # ---- H. yT = (act @ w2).T * coef ----
            for hc in range(HC):
                w2_sl = sbw.tile([P, II], bf16, tag="w2sl", bufs=3)
                iw2 = nc.sync.dma_start(
                    out=w2_sl[:], in_=w2[hc * P:(hc + 1) * P, :])
                if hc < 3:
                    w_gate_b.append(iw2)
                for goff, gn in groups:
                    gs = slice(goff, goff + gn)
                    pso = psg.tile([P, gn], f32, tag="mm2", name=f"pso{hc}_{goff}")
                    for ic in range(IC):
                        nc.tensor.matmul(
                            out=pso[:],
                            lhsT=w2_sl[:, ic * P:(ic + 1) * P],
                            rhs=actT[ic][:, gs],
                            start=(ic == 0), stop=(ic == IC - 1),
                        )
                    yt_sb = sbw.tile([P, gn], bf16, tag="yt", name=f"yt{hc}_{goff}")
                    nc.vector.tensor_mul(out=yt_sb[:], in0=pso[:], in1=cbc[:, gs])
                    nc.sync.dma_start(
                        out=o_yt[hc * P:(hc + 1) * P, gs], in_=yt_sb[:]
                    )

            # ---- outputs that are off the critical path ----
            nc.scalar.dma_start(out=o_cnt[:], in_=nf1[:])
            nc.scalar.dma_start(out=o_idx[:], in_=pk_i[:])

            # gate the leading weight loads behind the x-transpose stream;
            # the rest behind the token gathers (so gathers get HBM bandwidth)
            for wi_inst in w_gate:
                add_dep_helper(wi_inst.ins, last_x, sync=True,
                               reason="weight stream after router x stream")
                add_dep_helper(wi_inst.ins, last_x2, sync=True,
                               reason="weight stream after router x stream")
            last_gather = gather_insts[-1].ins
            for wi_inst in w_gate_b:
                add_dep_helper(wi_inst.ins, last_gather, sync=True,
                               reason="bulk weight stream after token gathers")

    nc.compile()
    return nc


def _get_built(cap):
    if cap not in _build_cache:
        _build_cache[cap] = _build(cap)
    return _build_cache[cap]


def _run(cap, hs, gate_w, w1s, w2s, w3s, trace=False):
    import ml_dtypes
    from concourse.bass_utils import run_bass_kernel_spmd

    nc = _get_built(cap)

    bf = ml_dtypes.bfloat16
    f8 = ml_dtypes.float8_e4m3
    x_hi = hs.astype(bf)
    x_lo = hs - x_hi.astype(np.float32)
    xth_np = np.ascontiguousarray(x_hi.T)
    xtl8_np = np.ascontiguousarray((x_lo * 2.0 ** 7).astype(f8).T)
    gw8_np = (gate_w * 2.0 ** 4).astype(f8)

    def block_w13(w):
        # [hc*128+p, icg*512+q] -> [icg*128+p, hc*512+q]
        return np.ascontiguousarray(
            w.astype(bf).reshape(8, 128, 8, 512).transpose(2, 1, 0, 3).reshape(1024, 4096))

    def block_w2(w):
        # [ic*128+p, hc*128+h] -> [hc*128+p, ic*128+h]
        return np.ascontiguousarray(
            w.astype(bf).reshape(32, 128, 8, 128).transpose(2, 1, 0, 3).reshape(1024, 4096))
    gw_hi = gate_w.astype(bf)
    gw_lo = (gate_w - gw_hi.astype(np.float32)).astype(bf)
    x_bf = np.ascontiguousarray(x_hi)
    oh_base = np.zeros((P, NT, E), np.float32)
    tokid1_np = (np.arange(NT)[None, :] * P + np.arange(P)[:, None] + 1.0).astype(np.float32)
    slotg_np = (np.arange(cap // 16)[None, :] * 16 + np.arange(16)[:, None]).astype(np.float32)
    ident_np = np.eye(P, dtype=np.float32)

    in_maps = []
    for c in range(NCORES):
        oh_c = oh_base.copy()
        oh_c[:, :, c] = 1.0
        in_maps.append({
            "xth": xth_np,
            "xtl8": xtl8_np,
            "x": x_bf,
            "gwh": gw_hi,
            "gwl": gw_lo,
            "gw8": gw8_np,
            "w1": block_w13(w1s[c]),
            "w3": block_w13(w3s[c]),
            "w2": block_w2(w2s[c]),
            "oh": oh_c.reshape(P, NT * E),
            "tokid1": tokid1_np,
            "slotg": slotg_np,
            "ident": ident_np,
        })

    res = run_bass_kernel_spmd(nc, in_maps, list(range(NCORES)), trace=trace)
    return res


def kernel(hidden_states, gate_w, w1s, w2s, w3s, _trace=False, _cap=560):
    hs = np.ascontiguousarray(np.asarray(hidden_states, dtype=np.float32))
    gate_w = np.ascontiguousarray(np.asarray(gate_w, dtype=np.float32))
    w1s = np.asarray(w1s, dtype=np.float32)
    w2s = np.asarray(w2s, dtype=np.float32)
    w3s = np.asarray(w3s, dtype=np.float32)

    cap = _cap
    while True:
        res = _run(cap, hs, gate_w, w1s, w2s, w3s, trace=_trace)
        counts = [int(res.results[c]["o_cnt"].ravel()[0]) for c in range(NCORES)]
        if max(counts) <= cap:
            break
        # capacity overflow (won't happen for sane routing): rebuild bigger
        cap = 2048 if max(counts) > 1024 else 1024

    out = np.zeros((T, H), dtype=np.float32)
    for c in range(NCORES):
        r = res.results[c]
        # columns are in sparse_gather-native order; invalid slots are
        # exactly zero (token 0, coef 0), so add.at over all columns is safe
        idx = np.asarray(r["o_idx"])
        y = np.ascontiguousarray(r["o_yt"].T).astype(np.float32)
        np.add.at(out, idx, y)
    kernel._last_results = res
    return out
